# revision 30
# baseline (speedup 1.0000x reference)
"""GCNConv (N=100000, E=1.6M, 128->64) on 8 Trainium2 NeuronCores.

Strategy (graph/edge parallel, per the sharding hint):
  out[i] = dis[i] * ( sum_{e: row_e = i, row!=col} dis[col_e] * h[col_e]
                      + dis[i] * h[i] )  + bias          (h = x @ W)
  using separability of the GCN edge weight w_e = dis[row] * dis[col].

Per core (SPMD, one static program, per-core data):
  The host pre-expands the per-edge source features into a per-core
  column stream xe [128, T_TOT] bf16, where token t's column is
  x[col_t] * dis[col_t] (or x[i] * dis[i]^2 for the synthetic self-loop
  token of node i, or zero for padding).  This is index-space
  duplication/permutation of the input (like the xt packing) - all
  O(E*F) math stays on device:
    stage 1 (expansion): msgs[t] = xe[:, t]^T @ W per 128-token group
            via PE matmuls (lhsT = xe block, rhs = W), psum -> bf16 SBUF
            via scalar-engine copies.
    stage 2 (scatter): destination windows of 128 nodes are distributed
            across cores (balanced by group count) as "slots"; tokens are
            grouped per slot.  A one-hot S[k, m] = (dest_rel_k == m) is
            built by batched DVE is_equal, and a PE matmul accumulates
            psum[128,64] += S.T @ msgs per group.
    flush:  out = psum * dis_dest + bias.
  The xe stream is fully affine (big DMA packets, no per-edge gather
  descriptors, no gpsimd software DGE).
Host does index-space preprocessing only (degree counts, edge
permutation/padding, layout packing); all O(E*F) math runs on device.
"""
import numpy as np
import ml_dtypes

P = 128
FIN, FOUT = 128, 64
N = 100000
N_CORES = 8
SB_SLOTS = 6             # slots (dest windows) per superblock
NW = (N + P - 1) // P    # 782 dest windows

BF16 = ml_dtypes.bfloat16
FP8 = ml_dtypes.float8_e4m3


def preprocess(x, edge_index, weight, bias):
    row = np.asarray(edge_index[0]).astype(np.int64)
    col = np.asarray(edge_index[1]).astype(np.int64)
    deg = np.bincount(row, minlength=N).astype(np.float32)
    with np.errstate(divide="ignore"):
        dis = deg ** np.float32(-0.5)
    n_inf = int(np.isinf(dis).sum())

    keep = row != col
    er = np.concatenate([row[keep], np.arange(N, dtype=np.int64)])
    # source column in xall is x[src]*dis[src] for both edge and self tokens:
    # the flush multiplies by dis[dest], giving dis_i*dis_c*h_c + dis_i^2*h_i
    esrc = np.concatenate([col[keep], np.arange(N, dtype=np.int64)])

    win = er // P
    cnt = np.bincount(win, minlength=NW)
    grp_w = -(-cnt // P)

    # rank-dealt window -> (core, slot) assignment: windows sorted by group
    # count; slot s holds ranks 8s..8s+7 (snake order across cores), so the
    # per-slot max over cores stays tight and cores stay balanced
    order = np.argsort(grp_w, kind="stable")
    S_SLOTS = -(-NW // N_CORES)
    slot_win = -np.ones((N_CORES, S_SLOTS), dtype=np.int64)
    core_of_win = np.zeros(NW, dtype=np.int32)
    for s in range(S_SLOTS):
        ws = order[s * N_CORES: (s + 1) * N_CORES]
        for j, w in enumerate(ws):
            c = j if s % 2 == 0 else len(ws) - 1 - j
            slot_win[c, s] = w
            core_of_win[w] = c

    # static per-slot group counts = max over cores
    B_s = np.zeros(S_SLOTS, dtype=np.int64)
    for c in range(N_CORES):
        for s in range(S_SLOTS):
            w = slot_win[c, s]
            if w >= 0:
                B_s[s] = max(B_s[s], grp_w[w])

    tok_off = np.zeros(S_SLOTS + 1, dtype=np.int64)
    tok_off[1:] = np.cumsum(B_s * P)
    T_TOT = int(tok_off[-1])
    G_TOT = T_TOT // P
    n_sb = -(-S_SLOTS // SB_SLOTS)
    sb_tok_off = np.zeros(n_sb + 1, dtype=np.int64)
    for isb in range(n_sb):
        sb_tok_off[isb] = tok_off[isb * SB_SLOTS]
    sb_tok_off[n_sb] = T_TOT

    ZERO_COL = N
    src_all = np.full((N_CORES, T_TOT), ZERO_COL, dtype=np.int64)
    dest_all = np.zeros((N_CORES, T_TOT), dtype=np.int16)

    slot_of_win = np.full(NW, -1, dtype=np.int64)
    for c in range(N_CORES):
        slot_of_win[:] = -1
        for s in range(S_SLOTS):
            w = slot_win[c, s]
            if w >= 0:
                slot_of_win[w] = s
        m = core_of_win[win] == c
        e_s = slot_of_win[win[m]]
        e_src = esrc[m]
        e_dr = (er[m] % P).astype(np.int16)
        sort = np.argsort(e_s, kind="stable")
        e_s, e_src, e_dr = e_s[sort], e_src[sort], e_dr[sort]
        change = np.flatnonzero(np.diff(e_s)) + 1
        starts = np.concatenate([[0], change])
        run_id = np.zeros(len(e_s), dtype=np.int64)
        run_id[change] = 1
        run_id = np.cumsum(run_id)
        within = np.arange(len(e_s)) - starts[run_id]
        pos = tok_off[e_s] + within
        src_all[c, pos] = e_src
        dest_all[c, pos] = e_dr

    # xall rows: [x*dis | zero], row-major for fast row gather
    xs = np.asarray(x, dtype=np.float32) * dis[:, None]
    if n_inf:
        xs = np.nan_to_num(xs, nan=0.0, posinf=0.0, neginf=0.0)
    xall = np.zeros((N + 1, FIN), dtype=BF16)
    xall[:N] = xs.astype(BF16)

    xe_dev = np.empty((N_CORES, FIN, T_TOT), dtype=BF16)
    for c in range(N_CORES):
        xe_dev[c] = np.ascontiguousarray(xall[src_all[c]].T)

    dest_dev = np.empty((N_CORES, 128, G_TOT), dtype=BF16)
    for c in range(N_CORES):
        dest_dev[c] = dest_all[c].reshape(G_TOT, 128).T.astype(BF16)

    dis_dev = np.zeros((N_CORES, 128, S_SLOTS), dtype=np.float32)
    for c in range(N_CORES):
        for s in range(S_SLOTS):
            w = slot_win[c, s]
            if w >= 0:
                lo = w * P
                hi = min(lo + P, N)
                dis_dev[c, : hi - lo, s] = dis[lo:hi]

    w_dev = np.asarray(weight, dtype=np.float32).astype(BF16)
    bias_dev = np.tile(np.asarray(bias, dtype=np.float32), (P, 1))
    iota = np.tile(np.arange(P, dtype=np.float32).astype(BF16), (P, 1))

    return dict(
        S_SLOTS=S_SLOTS, B_s=B_s, n_sb=n_sb, tok_off=tok_off,
        sb_tok_off=sb_tok_off, T_TOT=T_TOT, G_TOT=G_TOT,
        slot_win=slot_win, xe_dev=xe_dev, dest_dev=dest_dev, dis_dev=dis_dev,
        w_dev=w_dev, bias_dev=bias_dev, iota=iota, n_inf=n_inf,
    )


def build_bass(pp):
    import concourse.bacc as bacc
    import concourse.tile as tile
    from concourse import mybir

    dt = mybir.dt
    S_SLOTS, B_s = pp["S_SLOTS"], pp["B_s"]
    T_TOT, G_TOT, n_sb = pp["T_TOT"], pp["G_TOT"], pp["n_sb"]
    sb_tok_off, tok_off = pp["sb_tok_off"], pp["tok_off"]
    TSB_MAX = int(np.diff(sb_tok_off).max())
    GSB_MAX = TSB_MAX // P

    nc = bacc.Bacc("TRN2", target_bir_lowering=False, debug=False,
                   num_devices=N_CORES)
    xe_d = nc.dram_tensor("xe", [FIN, T_TOT], dt.bfloat16, kind="ExternalInput")
    w_d = nc.dram_tensor("w", [FIN, FOUT], dt.bfloat16, kind="ExternalInput")
    bias_d = nc.dram_tensor("bias", [P, FOUT], dt.float32, kind="ExternalInput")
    dest_d = nc.dram_tensor("dest", [P, G_TOT], dt.bfloat16, kind="ExternalInput")
    dis_d = nc.dram_tensor("dis", [P, S_SLOTS], dt.float32, kind="ExternalInput")
    iota_d = nc.dram_tensor("iota", [P, P], dt.bfloat16, kind="ExternalInput")
    out_d = nc.dram_tensor("out", [P, S_SLOTS * FOUT], dt.bfloat16,
                           kind="ExternalOutput")

    with tile.TileContext(nc) as tc:
        with tc.tile_pool(name="const", bufs=1) as cpool, \
             tc.tile_pool(name="xe", bufs=2) as xepool, \
             tc.tile_pool(name="msgs", bufs=2) as mpool, \
             tc.tile_pool(name="s", bufs=2) as spool, \
             tc.tile_pool(name="o", bufs=2) as opool, \
             tc.tile_pool(name="eps", bufs=3, space="PSUM") as epspool, \
             tc.tile_pool(name="ps", bufs=2, space="PSUM") as pspool:
            w_t = cpool.tile([FIN, FOUT], dt.bfloat16)
            nc.sync.dma_start(out=w_t[:], in_=w_d.ap())
            bias_t = cpool.tile([P, FOUT], dt.float32)
            nc.sync.dma_start(out=bias_t[:], in_=bias_d.ap())
            dis_t = cpool.tile([P, S_SLOTS], dt.float32)
            nc.sync.dma_start(out=dis_t[:], in_=dis_d.ap())
            iota_t = cpool.tile([P, P], dt.bfloat16)
            nc.sync.dma_start(out=iota_t[:], in_=iota_d.ap())
            # e-major materialized iota: iota_eg[p, e*G_PAD + g] = e.  Both
            # S-build operands then read dense (inner dim g step-1), which
            # enables the DVE 2x packed mode; a broadcast operand forces 1x.
            G_PAD = GSB_MAX
            iota_eg = cpool.tile([P, P * G_PAD], dt.bfloat16)
            nc.vector.tensor_copy(
                out=iota_eg[:].rearrange("p (e g) -> p e g", g=G_PAD),
                in_=iota_t[:].rearrange("p (e o) -> p e o", o=1)
                    .to_broadcast([P, P, G_PAD]))
            dest_t = cpool.tile([P, G_TOT + G_PAD], dt.bfloat16)
            nc.vector.memset(dest_t[:], 0)
            nc.sync.dma_start(out=dest_t[:, :G_TOT], in_=dest_d.ap())

            pending_flush = None
            for isb in range(n_sb):
                t0, t1 = int(sb_tok_off[isb]), int(sb_tok_off[isb + 1])
                T_SB = t1 - t0
                G_SB = T_SB // P
                g0 = t0 // P
                slots = range(isb * SB_SLOTS, min((isb + 1) * SB_SLOTS, S_SLOTS))
                ns = len(slots)

                xe_t = xepool.tile([128, TSB_MAX], dt.bfloat16, tag="xe")
                nc.sync.dma_start(out=xe_t[:, :T_SB], in_=xe_d.ap()[:, t0:t1])

                # stage 1: per-token projection msgs = xe_blk^T @ W
                msgs = mpool.tile([P, GSB_MAX * FOUT], dt.bfloat16, tag="m")
                for p16 in range(0, G_SB, 16):
                    pn = min(16, G_SB - p16)
                    eps = epspool.tile([P, 16 * FOUT], dt.float32, tag="eps")
                    for b in range(pn):
                        blk = p16 + b
                        nc.tensor.matmul(
                            out=eps[:, b * FOUT:(b + 1) * FOUT],
                            lhsT=xe_t[:, blk * P:(blk + 1) * P],
                            rhs=w_t[:],
                            start=True, stop=True,
                        )
                    nc.scalar.copy(out=msgs[:, p16 * FOUT:(p16 + pn) * FOUT],
                                   in_=eps[:, : pn * FOUT])

                # e-major one-hot S build: S[p, e*G_PAD + g] = (dest[p, g]==e).
                # dest reads are dense step-1 over g (stride-0 only on the
                # outer e dim) and iota_eg is a materialized dense tile, so
                # the op qualifies for the DVE 2x packed mode.  Chunked over
                # e to keep per-op size at the known-good level.
                s_t = spool.tile([P, P * G_PAD], dt.float8e4, tag="st")
                s3 = s_t[:].rearrange("p (e g) -> p e g", g=G_PAD)
                i3 = iota_eg[:].rearrange("p (e g) -> p e g", g=G_PAD)
                for ech in range(0, P, 32):
                    nc.vector.tensor_tensor(
                        out=s3[:, ech: ech + 32, :],
                        in0=dest_t[:, g0: g0 + G_PAD]
                            .rearrange("p (o g) -> p o g", o=1)
                            .to_broadcast([P, 32, G_PAD]),
                        in1=i3[:, ech: ech + 32, :],
                        op=mybir.AluOpType.is_equal,
                    )

                if pending_flush is not None:
                    pending_flush()
                    pending_flush = None

                # stage 2: scatter into per-slot psum columns
                out_sb = opool.tile([P, SB_SLOTS * FOUT], dt.bfloat16, tag="osb")
                ps = pspool.tile([P, SB_SLOTS * FOUT], dt.float32, tag="ps2")
                for si, s in enumerate(slots):
                    nb = int(B_s[s])
                    for g in range(nb):
                        blk = (int(tok_off[s]) - t0) // P + g
                        nc.tensor.matmul(
                            out=ps[:, si * FOUT: (si + 1) * FOUT],
                            lhsT=s3[:, :, blk],
                            rhs=msgs[:, blk * FOUT: (blk + 1) * FOUT],
                            start=(g == 0), stop=(g == nb - 1),
                        )
                # batched flush, deferred one superblock so the DVE queue
                # isn't blocked: S-build K+1 issues before flush K
                def flush(ps=ps, out_sb=out_sb, slots=slots, ns=ns):
                    nc.vector.tensor_tensor(
                        out=out_sb[:, : ns * FOUT]
                            .rearrange("p (g e) -> p g e", e=FOUT),
                        in0=ps[:, : ns * FOUT]
                            .rearrange("p (g e) -> p g e", e=FOUT),
                        in1=dis_t[:, slots.start: slots.start + ns]
                            .rearrange("p (g o) -> p g o", o=1)
                            .to_broadcast([P, ns, FOUT]),
                        op=mybir.AluOpType.mult,
                    )
                    nc.vector.tensor_tensor(
                        out=out_sb[:, : ns * FOUT]
                            .rearrange("p (g e) -> p g e", e=FOUT),
                        in0=out_sb[:, : ns * FOUT]
                            .rearrange("p (g e) -> p g e", e=FOUT),
                        in1=bias_t[:].rearrange("p (o e) -> p o e", o=1)
                            .to_broadcast([P, ns, FOUT]),
                        op=mybir.AluOpType.add,
                    )
                    nc.sync.dma_start(
                        out=out_d.ap()[:, slots.start * FOUT:
                                       (slots.start + ns) * FOUT],
                        in_=out_sb[:, : ns * FOUT])
                pending_flush = flush
            pending_flush()

    nc.compile()
    return nc


def assemble(pp, shards):
    out = np.zeros((N, FOUT), dtype=np.float32)
    for c in range(N_CORES):
        for s in range(pp["S_SLOTS"]):
            w = pp["slot_win"][c, s]
            if w < 0:
                continue
            lo = w * P
            hi = min(lo + P, N)
            out[lo:hi] = shards[c][: hi - lo, s * FOUT: (s + 1) * FOUT]
    return out


_CACHE = {}


def kernel(x, edge_index, weight, bias):
    from concourse import bass_utils

    pp = preprocess(x, edge_index, weight, bias)
    key = (pp["T_TOT"], pp["S_SLOTS"], pp["B_s"].tobytes())
    nc = _CACHE.get(key)
    if nc is None:
        nc = build_bass(pp)
        _CACHE[key] = nc

    in_maps = []
    for c in range(N_CORES):
        in_maps.append({
            "xe": pp["xe_dev"][c], "w": pp["w_dev"], "bias": pp["bias_dev"],
            "dest": pp["dest_dev"][c], "dis": pp["dis_dev"][c],
            "iota": pp["iota"],
        })
    res = bass_utils.run_bass_kernel_spmd(nc, in_maps,
                                          core_ids=list(range(N_CORES)))
    shards = [res.results[c]["out"] for c in range(N_CORES)]
    return assemble(pp, shards)


# revision 31
# speedup vs baseline: 1.4113x; 1.4113x over previous
"""GCNConv (N=100000, E=1.6M, 128->64) on 8 Trainium2 NeuronCores.

Strategy (graph/edge parallel, per the sharding hint):
  out[i] = dis[i] * ( sum_{e: row_e = i, row!=col} dis[col_e] * h[col_e]
                      + dis[i] * h[i] )  + bias          (h = x @ W)
  using separability of the GCN edge weight w_e = dis[row] * dis[col].

Per core (SPMD, one static program, per-core data):
  The host pre-expands the per-edge source features into a per-core
  column stream xe [128, T_TOT] bf16, where token t's column is
  x[col_t] * dis[col_t] (or x[i] * dis[i]^2 for the synthetic self-loop
  token of node i, or zero for padding).  This is index-space
  duplication/permutation of the input (like the xt packing) - all
  O(E*F) math stays on device:
    stage 1 (expansion): msgs[t] = xe[:, t]^T @ W per 128-token group
            via PE matmuls (lhsT = xe block, rhs = W), psum -> bf16 SBUF
            via scalar-engine copies.
    stage 2 (scatter): destination windows of 128 nodes are distributed
            across cores (balanced by group count) as "slots"; tokens are
            grouped per slot.  A one-hot S[k, m] = (dest_rel_k == m) is
            built by batched DVE is_equal, and a PE matmul accumulates
            psum[128,64] += S.T @ msgs per group.
    flush:  out = psum * dis_dest + bias.
  The xe stream is fully affine (big DMA packets, no per-edge gather
  descriptors, no gpsimd software DGE).
Host does index-space preprocessing only (degree counts, edge
permutation/padding, layout packing); all O(E*F) math runs on device.
"""
import numpy as np
import ml_dtypes

P = 128
FIN, FOUT = 128, 64
N = 100000
N_CORES = 8
SB_SLOTS = 6             # slots (dest windows) per superblock
NW = (N + P - 1) // P    # 782 dest windows

BF16 = ml_dtypes.bfloat16
FP8 = ml_dtypes.float8_e4m3


def preprocess(x, edge_index, weight, bias):
    row = np.asarray(edge_index[0]).astype(np.int64)
    col = np.asarray(edge_index[1]).astype(np.int64)
    deg = np.bincount(row, minlength=N).astype(np.float32)
    with np.errstate(divide="ignore"):
        dis = deg ** np.float32(-0.5)
    n_inf = int(np.isinf(dis).sum())

    keep = row != col
    er = np.concatenate([row[keep], np.arange(N, dtype=np.int64)])
    # source column in xall is x[src]*dis[src] for both edge and self tokens:
    # the flush multiplies by dis[dest], giving dis_i*dis_c*h_c + dis_i^2*h_i
    esrc = np.concatenate([col[keep], np.arange(N, dtype=np.int64)])

    win = er // P
    cnt = np.bincount(win, minlength=NW)
    grp_w = -(-cnt // P)

    # rank-dealt window -> (core, slot) assignment: windows sorted by group
    # count; slot s holds ranks 8s..8s+7 (snake order across cores), so the
    # per-slot max over cores stays tight and cores stay balanced
    order = np.argsort(grp_w, kind="stable")
    S_SLOTS = -(-NW // N_CORES)
    slot_win = -np.ones((N_CORES, S_SLOTS), dtype=np.int64)
    core_of_win = np.zeros(NW, dtype=np.int32)
    for s in range(S_SLOTS):
        ws = order[s * N_CORES: (s + 1) * N_CORES]
        for j, w in enumerate(ws):
            c = j if s % 2 == 0 else len(ws) - 1 - j
            slot_win[c, s] = w
            core_of_win[w] = c

    # static per-slot group counts = max over cores
    B_s = np.zeros(S_SLOTS, dtype=np.int64)
    for c in range(N_CORES):
        for s in range(S_SLOTS):
            w = slot_win[c, s]
            if w >= 0:
                B_s[s] = max(B_s[s], grp_w[w])

    tok_off = np.zeros(S_SLOTS + 1, dtype=np.int64)
    tok_off[1:] = np.cumsum(B_s * P)
    T_TOT = int(tok_off[-1])
    G_TOT = T_TOT // P
    n_sb = -(-S_SLOTS // SB_SLOTS)
    sb_tok_off = np.zeros(n_sb + 1, dtype=np.int64)
    for isb in range(n_sb):
        sb_tok_off[isb] = tok_off[isb * SB_SLOTS]
    sb_tok_off[n_sb] = T_TOT

    ZERO_COL = N
    src_all = np.full((N_CORES, T_TOT), ZERO_COL, dtype=np.int64)
    dest_all = np.zeros((N_CORES, T_TOT), dtype=np.int16)

    slot_of_win = np.full(NW, -1, dtype=np.int64)
    for c in range(N_CORES):
        slot_of_win[:] = -1
        for s in range(S_SLOTS):
            w = slot_win[c, s]
            if w >= 0:
                slot_of_win[w] = s
        m = core_of_win[win] == c
        e_s = slot_of_win[win[m]]
        e_src = esrc[m]
        e_dr = (er[m] % P).astype(np.int16)
        sort = np.argsort(e_s, kind="stable")
        e_s, e_src, e_dr = e_s[sort], e_src[sort], e_dr[sort]
        change = np.flatnonzero(np.diff(e_s)) + 1
        starts = np.concatenate([[0], change])
        run_id = np.zeros(len(e_s), dtype=np.int64)
        run_id[change] = 1
        run_id = np.cumsum(run_id)
        within = np.arange(len(e_s)) - starts[run_id]
        pos = tok_off[e_s] + within
        src_all[c, pos] = e_src
        dest_all[c, pos] = e_dr

    # xall rows: [x*dis | zero], row-major for fast row gather
    xs = np.asarray(x, dtype=np.float32) * dis[:, None]
    if n_inf:
        xs = np.nan_to_num(xs, nan=0.0, posinf=0.0, neginf=0.0)
    xall = np.zeros((N + 1, FIN), dtype=BF16)
    xall[:N] = xs.astype(BF16)

    xe_dev = np.empty((N_CORES, FIN, T_TOT), dtype=BF16)
    for c in range(N_CORES):
        xe_dev[c] = np.ascontiguousarray(xall[src_all[c]].T)

    dest_dev = np.empty((N_CORES, 128, G_TOT), dtype=BF16)
    for c in range(N_CORES):
        dest_dev[c] = dest_all[c].reshape(G_TOT, 128).T.astype(BF16)

    dis_dev = np.zeros((N_CORES, 128, S_SLOTS), dtype=np.float32)
    for c in range(N_CORES):
        for s in range(S_SLOTS):
            w = slot_win[c, s]
            if w >= 0:
                lo = w * P
                hi = min(lo + P, N)
                dis_dev[c, : hi - lo, s] = dis[lo:hi]

    w_dev = np.asarray(weight, dtype=np.float32).astype(BF16)
    bias_dev = np.tile(np.asarray(bias, dtype=np.float32), (P, 1))
    iota = np.tile(np.arange(P, dtype=np.float32).astype(BF16), (P, 1))

    return dict(
        S_SLOTS=S_SLOTS, B_s=B_s, n_sb=n_sb, tok_off=tok_off,
        sb_tok_off=sb_tok_off, T_TOT=T_TOT, G_TOT=G_TOT,
        slot_win=slot_win, xe_dev=xe_dev, dest_dev=dest_dev, dis_dev=dis_dev,
        w_dev=w_dev, bias_dev=bias_dev, iota=iota, n_inf=n_inf,
    )


def build_bass(pp):
    import concourse.bacc as bacc
    import concourse.tile as tile
    from concourse import mybir

    dt = mybir.dt
    S_SLOTS, B_s = pp["S_SLOTS"], pp["B_s"]
    T_TOT, G_TOT, n_sb = pp["T_TOT"], pp["G_TOT"], pp["n_sb"]
    sb_tok_off, tok_off = pp["sb_tok_off"], pp["tok_off"]
    TSB_MAX = int(np.diff(sb_tok_off).max())
    GSB_MAX = TSB_MAX // P

    nc = bacc.Bacc("TRN2", target_bir_lowering=False, debug=False,
                   num_devices=N_CORES)
    xe_d = nc.dram_tensor("xe", [FIN, T_TOT], dt.bfloat16, kind="ExternalInput")
    w_d = nc.dram_tensor("w", [FIN, FOUT], dt.bfloat16, kind="ExternalInput")
    bias_d = nc.dram_tensor("bias", [P, FOUT], dt.float32, kind="ExternalInput")
    dest_d = nc.dram_tensor("dest", [P, G_TOT], dt.bfloat16, kind="ExternalInput")
    dis_d = nc.dram_tensor("dis", [P, S_SLOTS], dt.float32, kind="ExternalInput")
    iota_d = nc.dram_tensor("iota", [P, P], dt.bfloat16, kind="ExternalInput")
    out_d = nc.dram_tensor("out", [P, S_SLOTS * FOUT], dt.bfloat16,
                           kind="ExternalOutput")

    with tile.TileContext(nc) as tc:
        with tc.tile_pool(name="const", bufs=1) as cpool, \
             tc.tile_pool(name="xe", bufs=2) as xepool, \
             tc.tile_pool(name="msgs", bufs=2) as mpool, \
             tc.tile_pool(name="s", bufs=2) as spool, \
             tc.tile_pool(name="o", bufs=2) as opool, \
             tc.tile_pool(name="eps", bufs=3, space="PSUM") as epspool, \
             tc.tile_pool(name="ps", bufs=2, space="PSUM") as pspool:
            w_t = cpool.tile([FIN, FOUT], dt.bfloat16)
            nc.sync.dma_start(out=w_t[:], in_=w_d.ap())
            bias_t = cpool.tile([P, FOUT], dt.float32)
            nc.sync.dma_start(out=bias_t[:], in_=bias_d.ap())
            dis_t = cpool.tile([P, S_SLOTS], dt.float32)
            nc.sync.dma_start(out=dis_t[:], in_=dis_d.ap())
            iota_t = cpool.tile([P, P], dt.bfloat16)
            nc.sync.dma_start(out=iota_t[:], in_=iota_d.ap())
            # e-major materialized iota: iota_eg[p, e*G_PAD + g] = e.  Both
            # S-build operands then read dense (inner dim g step-1), which
            # enables the DVE 2x packed mode; a broadcast operand forces 1x.
            G_PAD = GSB_MAX
            iota_eg = cpool.tile([P, P * G_PAD], dt.bfloat16)
            nc.vector.tensor_copy(
                out=iota_eg[:].rearrange("p (e g) -> p e g", g=G_PAD),
                in_=iota_t[:].rearrange("p (e o) -> p e o", o=1)
                    .to_broadcast([P, P, G_PAD]))
            dest_t = cpool.tile([P, G_TOT + G_PAD], dt.bfloat16)
            nc.vector.memset(dest_t[:], 0)
            nc.sync.dma_start(out=dest_t[:, :G_TOT], in_=dest_d.ap())

            pending_flush = None
            for isb in range(n_sb):
                t0, t1 = int(sb_tok_off[isb]), int(sb_tok_off[isb + 1])
                T_SB = t1 - t0
                G_SB = T_SB // P
                g0 = t0 // P
                slots = range(isb * SB_SLOTS, min((isb + 1) * SB_SLOTS, S_SLOTS))
                ns = len(slots)

                xe_t = xepool.tile([128, TSB_MAX], dt.bfloat16, tag="xe")
                if isb == 0:
                    # split the first slab so the pipeline fills sooner
                    h = (G_SB // 2) * P
                    nc.sync.dma_start(out=xe_t[:, :h], in_=xe_d.ap()[:, t0:t0 + h])
                    nc.sync.dma_start(out=xe_t[:, h:T_SB],
                                      in_=xe_d.ap()[:, t0 + h:t1])
                else:
                    nc.sync.dma_start(out=xe_t[:, :T_SB], in_=xe_d.ap()[:, t0:t1])

                # stage 1: per-token projection msgs = xe_blk^T @ W
                msgs = mpool.tile([P, GSB_MAX * FOUT], dt.bfloat16, tag="m")
                for p16 in range(0, G_SB, 16):
                    pn = min(16, G_SB - p16)
                    eps = epspool.tile([P, 16 * FOUT], dt.float32, tag="eps")
                    for b in range(pn):
                        blk = p16 + b
                        nc.tensor.matmul(
                            out=eps[:, b * FOUT:(b + 1) * FOUT],
                            lhsT=xe_t[:, blk * P:(blk + 1) * P],
                            rhs=w_t[:],
                            start=True, stop=True,
                        )
                    nc.scalar.copy(out=msgs[:, p16 * FOUT:(p16 + pn) * FOUT],
                                   in_=eps[:, : pn * FOUT])

                # e-major one-hot S build: S[p, e*G_PAD + g] = (dest[p, g]==e).
                # dest reads are dense step-1 over g (stride-0 only on the
                # outer e dim) and iota_eg is a materialized dense tile, so
                # the op qualifies for the DVE 2x packed mode.  Chunked over
                # e to keep per-op size at the known-good level.
                s_t = spool.tile([P, P * G_PAD], dt.bfloat16, tag="st")
                s3 = s_t[:].rearrange("p (e g) -> p e g", g=G_PAD)
                i3 = iota_eg[:].rearrange("p (e g) -> p e g", g=G_PAD)
                for ech in range(0, P, 32):
                    nc.vector.tensor_tensor(
                        out=s3[:, ech: ech + 32, :],
                        in0=dest_t[:, g0: g0 + G_PAD]
                            .rearrange("p (o g) -> p o g", o=1)
                            .to_broadcast([P, 32, G_PAD]),
                        in1=i3[:, ech: ech + 32, :],
                        op=mybir.AluOpType.is_equal,
                    )

                if pending_flush is not None:
                    pending_flush()
                    pending_flush = None

                # stage 2: scatter into per-slot psum columns
                out_sb = opool.tile([P, SB_SLOTS * FOUT], dt.bfloat16, tag="osb")
                ps = pspool.tile([P, SB_SLOTS * FOUT], dt.float32, tag="ps2")
                for si, s in enumerate(slots):
                    nb = int(B_s[s])
                    for g in range(nb):
                        blk = (int(tok_off[s]) - t0) // P + g
                        nc.tensor.matmul(
                            out=ps[:, si * FOUT: (si + 1) * FOUT],
                            lhsT=s3[:, :, blk],
                            rhs=msgs[:, blk * FOUT: (blk + 1) * FOUT],
                            start=(g == 0), stop=(g == nb - 1),
                        )
                # batched flush, deferred one superblock so the DVE queue
                # isn't blocked: S-build K+1 issues before flush K
                def flush(ps=ps, out_sb=out_sb, slots=slots, ns=ns):
                    nc.vector.tensor_tensor(
                        out=out_sb[:, : ns * FOUT]
                            .rearrange("p (g e) -> p g e", e=FOUT),
                        in0=ps[:, : ns * FOUT]
                            .rearrange("p (g e) -> p g e", e=FOUT),
                        in1=dis_t[:, slots.start: slots.start + ns]
                            .rearrange("p (g o) -> p g o", o=1)
                            .to_broadcast([P, ns, FOUT]),
                        op=mybir.AluOpType.mult,
                    )
                    nc.vector.tensor_tensor(
                        out=out_sb[:, : ns * FOUT]
                            .rearrange("p (g e) -> p g e", e=FOUT),
                        in0=out_sb[:, : ns * FOUT]
                            .rearrange("p (g e) -> p g e", e=FOUT),
                        in1=bias_t[:].rearrange("p (o e) -> p o e", o=1)
                            .to_broadcast([P, ns, FOUT]),
                        op=mybir.AluOpType.add,
                    )
                    nc.sync.dma_start(
                        out=out_d.ap()[:, slots.start * FOUT:
                                       (slots.start + ns) * FOUT],
                        in_=out_sb[:, : ns * FOUT])
                pending_flush = flush
            pending_flush()

    nc.compile()
    return nc


def assemble(pp, shards):
    out = np.zeros((N, FOUT), dtype=np.float32)
    for c in range(N_CORES):
        for s in range(pp["S_SLOTS"]):
            w = pp["slot_win"][c, s]
            if w < 0:
                continue
            lo = w * P
            hi = min(lo + P, N)
            out[lo:hi] = shards[c][: hi - lo, s * FOUT: (s + 1) * FOUT]
    return out


_CACHE = {}


def kernel(x, edge_index, weight, bias):
    from concourse import bass_utils

    pp = preprocess(x, edge_index, weight, bias)
    key = (pp["T_TOT"], pp["S_SLOTS"], pp["B_s"].tobytes())
    nc = _CACHE.get(key)
    if nc is None:
        nc = build_bass(pp)
        _CACHE[key] = nc

    in_maps = []
    for c in range(N_CORES):
        in_maps.append({
            "xe": pp["xe_dev"][c], "w": pp["w_dev"], "bias": pp["bias_dev"],
            "dest": pp["dest_dev"][c], "dis": pp["dis_dev"][c],
            "iota": pp["iota"],
        })
    res = bass_utils.run_bass_kernel_spmd(nc, in_maps,
                                          core_ids=list(range(N_CORES)))
    shards = [res.results[c]["out"] for c in range(N_CORES)]
    return assemble(pp, shards)
